# revision 23
# baseline (speedup 1.0000x reference)
"""Trainium2 Bass kernel for nn_AttenPool_22917945491863.

Mathematical reduction: in the reference, ``attn`` is softmaxed over axis 3
and then summed over that same axis — the sum of a softmax over its own axis
is exactly 1, so the whole query branch (2 convs, BN, ReLU, LayerNorm,
softmax) collapses to ``a = ones``. The remaining computation

    out = sumpool4x4((1-alpha) * (conv3x3(bn(x), wv) + bv) + alpha * x)

is a 6x6 stride-4 convolution over zero-padded x (sumpool of a 3x3 conv is a
6x6 stride-4 conv with summed taps; the BN scale folds into the weights; the
BN shift and conv bias fold into a precomputed per-output-position bias map;
the alpha*x sum-pool folds in as a depthwise component on the central 4x4
taps).

Device mapping (8 cores, batch-parallel, 2 samples each):
  - x is pre-shuffled on the host into a zero-padded h-parity, phase-major
    column layout [128, 65*132]: partition p holds channel (p % 64);
    partitions 0-63 hold even padded rows, 64-127 odd padded rows; padded
    col c sits at (c%4)*33 + c//4 within a row so each tap's 32 stride-4
    columns are contiguous in SBUF. Each matmul contracts over K=128 =
    64 channels x 2 vertically-adjacent taps.
  - The 36 conv taps become 18 tap-pair matmuls [K=128, M=64, N=512]
    (dtype float32r: fp32 storage, reduced-mantissa multiply at ~2x fp32
    speed) accumulated in PSUM; two N=512 output tiles per sample.
  - Raw engine blocks with manual semaphores (no Tile framework): Sync
    streams the x chunks, ACT loads weights/bias and drains outputs,
    PE runs the 72 matmuls, DVE adds the bias map from PSUM.
"""

import numpy as np

B, C, H, W = 16, 64, 128, 128
NCORES = 8
BPC = B // NCORES  # samples per core
OH = OW = 32  # output spatial
WPAD = 132  # padded row length: stored phase-major as [4 phases][33 cols]
NROW = 65  # padded rows per parity block
EPS = 1e-5
NT = 2 * BPC  # output tiles (sample x half)

_PROGRAM_CACHE = {}


def _build_program():
    import concourse.bacc as bacc
    import concourse.bass as bass
    import concourse.mybir as mybir

    class _NoBarrierBlock(bass.BassBlock):
        """BassBlock whose exit drains each used engine but skips the
        all-engine EVSEM butterfly barrier (~7.5us). The NEFF prologue's
        semaphore RANGE_CLEAR re-initializes sems on every execution, and
        the kernel's own osem wait guarantees outputs landed, so the
        cross-engine barrier adds nothing here."""

        def __exit__(self, exc_type, exc_val, exc_tb):
            if exc_type is not None:
                return
            for engine, last_body in self.last_body.items():
                with self.bass.body(last_body, parent=self.bass.cur_bb,
                                    allow_existing_parent=True):
                    engine.br(self.end_bb)
            self.bass.switch_bb(self.end_bb)
            gpsimd_type = self.bass.gpsimd.engine
            for eng_type, eng in self.bass.engines.items():
                if eng_type == gpsimd_type:
                    continue
                d = mybir.InstDrain(
                    name=self.bass.get_next_instruction_name(),
                    ins=[], outs=[], bass_is_fusable=False)
                d.engine = eng_type
                eng.add_instruction(d)

    f32 = mybir.dt.float32
    # fp32r: fp32 storage, reduced-mantissa matmul (measured rel err 1.8e-4
    # on [128,128]x[128,512] vs f64). The whole producer chain must be
    # declared float32r for the BIR verifier; HW accepts unrounded fp32.
    xdt = mybir.dt.float32r

    nc = bacc.Bacc("TRN2", target_bir_lowering=False, debug=False,
                   num_devices=NCORES)
    xp = nc.dram_tensor("xp", [BPC, 128, NROW * WPAD], xdt,
                        kind="ExternalInput").ap()
    w_in = nc.dram_tensor("w", [128, 18 * 64], xdt, kind="ExternalInput").ap()
    ab_in = nc.dram_tensor("abias", [C, OH * OW], f32,
                           kind="ExternalInput").ap()
    out = nc.dram_tensor("out", [BPC, C, OH * OW], f32,
                         kind="ExternalOutput").ap()

    x2 = [nc.alloc_sbuf_tensor(f"x2_{b}", [128, NROW * WPAD], xdt).ap()
          for b in range(BPC)]
    w_sb = nc.alloc_sbuf_tensor("w_sb", [128, 18 * 64], xdt).ap()
    ab_sb = nc.alloc_sbuf_tensor("ab_sb", [C, OH * OW], f32).ap()
    # Fine-grained pipeline: 4 x-chunks and 4 N=256 output tiles (8 ph rows
    # each) per sample. A tile over ph in [p0, p0+8) reads padded free rows
    # [2*p0, 2*p0+17], so tile j of a sample is gated on that sample's
    # chunks 0..j. 8 tiles use exactly the 8 PSUM banks.
    CHUNKS = [[(0, 18), (18, 34), (34, 50), (50, NROW)] for _ in range(BPC)]
    # (sample, ph0, nph, gating chunk sem index + 1)
    TILES = [(b, 8 * j, 8, 4 * b + j + 1)
             for b in range(BPC) for j in range(4)]
    NTILE = len(TILES)
    ob = [nc.alloc_sbuf_tensor(f"ob_{t}", [C, 32 * nph], f32).ap()
          for t, (_, _, nph, _) in enumerate(TILES)]
    ps = [nc.alloc_psum_tensor(f"ps_{t}", [C, 32 * nph], f32).ap()
          for t, (_, _, nph, _) in enumerate(TILES)]

    # One semaphore per gating DMA: with several DMAs in flight on one ring
    # a shared counter can hit 16 via a mix of transfers (engine lanes run
    # unevenly), so a >=16 wait on a shared sem does NOT mean "my transfer
    # landed". A dedicated sem at 16 does — and per-engine FIFO order then
    # implies every earlier transfer on the ring is complete as well.
    wsem = nc.alloc_semaphore("wsem")   # w landed (=> nothing else needed)
    absem = nc.alloc_semaphore("absem")  # abias landed (never waited on)
    csem = [nc.alloc_semaphore(f"csem{i}") for i in range(4 * BPC)]  # chunks
    mmsem = nc.alloc_semaphore("mmsem")  # per-tile matmul group done
    vsem = nc.alloc_semaphore("vsem")   # per-tile bias add done
    osem = nc.alloc_semaphore("osem")   # output DMAs landed

    with _NoBarrierBlock(nc, "main") as block:

        @block.sync
        def _(sync):
            # single HWDGE FIFO, ordered by consumption: weights/bias first
            # (small, gate the PE), then the x chunks, then the outputs
            sync.dma_start(out=w_sb[:], in_=w_in[:]).then_inc(wsem, 16)
            ci = 0
            for b in range(BPC):
                for r0, r1 in CHUNKS[b]:
                    sync.dma_start(
                        out=x2[b][:, r0 * WPAD:r1 * WPAD],
                        in_=xp[b, :, r0 * WPAD:r1 * WPAD],
                    ).then_inc(csem[ci], 16)
                    ci += 1
                if b == 0:
                    # abias rides mid-stream: its completion precedes the
                    # first DVE add (gated behind sample-1 chunks' matmuls
                    # is not needed — DVE add 0 waits mmsem which implies
                    # sample-0 chunk 3 landed, which implies abias landed).
                    sync.dma_start(out=ab_sb[:], in_=ab_in[:]).then_inc(
                        absem, 16)
            for t, (b, p0, nph, _) in enumerate(TILES):
                sync.wait_ge(vsem, t + 1)
                sync.dma_start(
                    out=out[b, :, p0 * 32:(p0 + nph) * 32],
                    in_=ob[t][:],
                ).then_inc(osem, 16)
            sync.wait_ge(osem, 16 * NTILE)

        @block.tensor
        def _(tensor):
            tensor.wait_ge(wsem, 16)
            for t, (b, p0, nph, nchunk) in enumerate(TILES):
                tensor.wait_ge(csem[nchunk - 1], 16)
                v = x2[b].rearrange("p (r f c) -> p r f c", f=4, c=33)
                for i in range(18):
                    a, sw = divmod(i, 6)
                    r0 = 2 * p0 + a
                    rhs = v[:, r0: r0 + 2 * nph - 1: 2, sw % 4,
                            sw // 4: sw // 4 + 32]
                    mm = tensor.matmul(ps[t][:], w_sb[:, i * 64:(i + 1) * 64],
                                       rhs, start=(i == 0), stop=(i == 17))
                    if i == 17:
                        mm.then_inc(mmsem, 1)

        @block.vector
        def _(vector):
            vector.wait_ge(absem, 16)
            for t, (b, p0, nph, _) in enumerate(TILES):
                vector.wait_ge(mmsem, t + 1)
                vector.tensor_add(
                    ob[t][:], ps[t][:],
                    ab_sb[:, p0 * 32:(p0 + nph) * 32],
                ).then_inc(vsem, 1)

    nc.compile()
    return nc


def _host_precompute(inputs):
    """Fold BN/alpha/bias into 6x6 stride-4 conv weights + bias map (f64)."""
    g0 = np.asarray(inputs["g0"], np.float64)
    b0 = np.asarray(inputs["b0"], np.float64)
    m0 = np.asarray(inputs["m0"], np.float64)
    v0 = np.asarray(inputs["v0"], np.float64)
    wv = np.asarray(inputs["wv"], np.float64)
    bv = np.asarray(inputs["bv"], np.float64)
    alpha = float(np.asarray(inputs["alpha"]))

    s0 = g0 / np.sqrt(v0 + EPS)
    t0 = b0 - m0 * s0

    # W'[o,c,sh,sw] = sum of 3x3 taps t with s - t in [0,4)^2
    Wp = np.zeros((C, C, 6, 6))
    for sh in range(6):
        for sw in range(6):
            th0, th1 = max(0, sh - 3), min(3, sh + 1)
            tw0, tw1 = max(0, sw - 3), min(3, sw + 1)
            Wp[:, :, sh, sw] = wv[:, :, th0:th1, tw0:tw1].sum(axis=(2, 3))

    W_final = (1.0 - alpha) * Wp * s0[None, :, None, None]
    idx = np.arange(C)
    for sh in range(1, 5):
        for sw in range(1, 5):
            W_final[idx, idx, sh, sw] += alpha

    # bias map: contribution of the BN shift t0 through the conv (with
    # zero-padding mask) plus conv bias, scaled by (1-alpha)
    Rm = np.zeros((OH, 6))
    for p in range(OH):
        for s in range(6):
            if 0 <= 4 * p + s - 1 < H:
                Rm[p, s] = 1.0
    A0 = np.einsum("ocuv,pu,qv,c->opq", Wp, Rm, Rm, t0)
    Abias = (1.0 - alpha) * (A0 + 16.0 * bv[:, None, None])

    # lhsT tap-pair layout: pair i = (a, sw), rows 0-63 = tap (2a, sw),
    # rows 64-127 = tap (2a+1, sw); [k, i*64 + m] with k=ci, m=co
    W18 = np.zeros((128, 18 * 64))
    for i in range(18):
        a, sw = divmod(i, 6)
        W18[0:64, i * 64:(i + 1) * 64] = W_final[:, :, 2 * a, sw].T
        W18[64:128, i * 64:(i + 1) * 64] = W_final[:, :, 2 * a + 1, sw].T

    return W18, Abias.reshape(C, OH * OW)


def _host_shuffle_x(x):
    """Zero-padded h-parity, phase-major-column layout [B, 128, NROW*WPAD].

    Partition p < 64: channel p, even padded rows (pad row 2*r -> h=2r-1);
    partition p >= 64: channel p-64, odd padded rows (pad row 2*r+1 -> h=2r).
    Padded col c (data cols 1..128, zeros at 0/129/130/131) is stored at
    row offset (c%4)*33 + c//4 so stride-4 tap reads are contiguous.
    """
    xpad = np.zeros((B, 128, NROW, WPAD), np.float32)
    xpad[:, 0:64, 1:65, 1:129] = x[:, :, 1::2, :]
    xpad[:, 64:128, 0:64, 1:129] = x[:, :, 0::2, :]
    # c = cc*4 + phase -> phase-major [4][33]
    xph = xpad.reshape(B, 128, NROW, 33, 4).transpose(0, 1, 2, 4, 3)
    return np.ascontiguousarray(xph).reshape(B, 128, NROW * WPAD)


def kernel(**inputs):
    from concourse.bass_utils import run_bass_kernel_spmd

    x = np.asarray(inputs["x"], np.float32)
    W18, Abias = _host_precompute(inputs)
    w_host = W18.astype(np.float32)
    ab_host = Abias.astype(np.float32)
    xp = _host_shuffle_x(x)

    if "nc" not in _PROGRAM_CACHE:
        _PROGRAM_CACHE["nc"] = _build_program()
    nc = _PROGRAM_CACHE["nc"]

    in_maps = [
        {"xp": xp[i * BPC:(i + 1) * BPC], "w": w_host, "abias": ab_host}
        for i in range(NCORES)
    ]
    res = run_bass_kernel_spmd(nc, in_maps, list(range(NCORES)))
    out = np.concatenate(
        [res.results[i]["out"].reshape(BPC, C, OH, OW) for i in range(NCORES)],
        axis=0,
    )
    return np.ascontiguousarray(out.astype(np.float32))
